# revision 1
# baseline (speedup 1.0000x reference)
"""Trainium2 Bass kernel for relative-position multi-head attention
(Transformer-XL style), sharded over 8 NeuronCores by head (2 heads/core)
with row-parallel output projection (partial sums reduced on host).

Math (per core c, d-slice = rows 128c..128c+128 of the projection space):
  qT = Wq[ds] @ Q.T          (128, L)   [+ bq]
  kT = Wk[ds] @ K.T + bk     (128, L)
  v  = V @ Wv[ds].T          (L, 128)   [bv folded on host]
  per head h (64-row slice of the 128):
    S  = ((q+u)/8).T @ k  +  shift(((q+v)/8).T @ F.T)   (L, L)
    P  = exp(S);  out = (P @ v_h) / P.sum(1)
  O_part = concat(out).T-contraction with Wo[:, ds]  ->  O^T (1024, L) f32
Host: out = (sum_c O_part).T + bo + bv @ Wo.T

The relative-shift uses a banded matmul (width 2176 per 128-row l-tile)
against F = flip(rel_emb), followed by a diagonal SBUF->SBUF DMA with
accum_op=add: row p of the shifted band starts at byte offset (127-p)*2,
expressed as access pattern [[W-1, 128], [1, 2048]] at offset 127.
"""

import math
import numpy as np
import ml_dtypes

import concourse.bass as bass
import concourse.bacc as bacc
import concourse.mybir as mybir
import concourse.tile as tile
from concourse.bass_utils import run_bass_kernel_spmd
from contextlib import ExitStack

BF16 = mybir.dt.bfloat16
F32 = mybir.dt.float32
AF = mybir.ActivationFunctionType

L = 2048          # sequence length
D = 1024          # model dim
DK = 64           # head dim
NH = 16           # total heads
NCORES = 8
DH = 128          # per-core projection slice (2 heads * 64)
LT = 128          # l-tile rows
NLT = L // LT     # 16 l-tiles
MC = 512          # m-chunk for AC matmuls
BW = 2176         # band width per l-tile (2175 rounded up to 17*128)
LC = 512          # l-chunk for PV/Wo stage
NLC = L // LC     # 4


def _build_module():
    nc = bacc.Bacc("TRN2", target_bir_lowering=False, debug=False,
                   enable_asserts=False, num_devices=NCORES)

    # ---- DRAM I/O ----
    d_qt = nc.dram_tensor("qt", (D, L), BF16, kind="ExternalInput")
    d_kt = nc.dram_tensor("kt", (D, L), BF16, kind="ExternalInput")
    d_vt = nc.dram_tensor("vt", (D, L), BF16, kind="ExternalInput")
    d_wqt = nc.dram_tensor("wqt", (D, DH), BF16, kind="ExternalInput")
    d_wkt = nc.dram_tensor("wkt", (D, DH), BF16, kind="ExternalInput")
    d_wvt = nc.dram_tensor("wvt", (D, DH), BF16, kind="ExternalInput")
    d_wot = nc.dram_tensor("wot", (DH, D), BF16, kind="ExternalInput")
    d_ft = nc.dram_tensor("ft", (DK, 4096), BF16, kind="ExternalInput")
    d_ub = nc.dram_tensor("ubias", (DH, 1), F32, kind="ExternalInput")
    d_vb = nc.dram_tensor("vbias", (DH, 1), F32, kind="ExternalInput")
    d_kb = nc.dram_tensor("kbias", (DH, 1), F32, kind="ExternalInput")
    d_ones2 = nc.dram_tensor("ones2", (2, DK), F32, kind="ExternalInput")
    d_out = nc.dram_tensor("opart", (D, L), F32, kind="ExternalOutput")

    with tile.TileContext(nc) as tc, ExitStack() as ctx:
        const = ctx.enter_context(tc.tile_pool(name="const", bufs=1))
        persist = ctx.enter_context(tc.tile_pool(name="persist", bufs=1))

        # ---- persistent SBUF loads ----
        ft_s = const.tile([128, 4096], BF16)
        nc.sync.dma_start(ft_s[DK:128, :], d_ft[:])
        wqt_s = const.tile([128, 8 * DH], BF16)
        wkt_s = const.tile([128, 8 * DH], BF16)
        wvt_s = const.tile([128, 8 * DH], BF16)
        for dd, ss in ((d_wqt, wqt_s), (d_wkt, wkt_s), (d_wvt, wvt_s)):
            nc.sync.dma_start(
                ss[:].rearrange("p (a d) -> p a d", a=8),
                dd[:].rearrange("(a p) d -> p a d", p=128))
        wot_s = const.tile([DH, D], BF16)
        nc.sync.dma_start(wot_s[:], d_wot[:])
        ub_s = const.tile([DH, 1], F32)
        vb_s = const.tile([DH, 1], F32)
        kb_s = const.tile([DH, 1], F32)
        nc.sync.dma_start(ub_s[:], d_ub[:])
        nc.sync.dma_start(vb_s[:], d_vb[:])
        nc.sync.dma_start(kb_s[:], d_kb[:])

        # per-head projection outputs (64 partitions each)
        quv = [persist.tile([128, L], BF16, tag=f"quv{h}", name=f"quv{h}") for h in range(2)]
        kth = [persist.tile([DK, L], BF16, tag=f"kth{h}", name=f"kth{h}") for h in range(2)]
        # v tiles per (head, m-tile): (128, 65) with ones in col 64
        vmt = [[persist.tile([128, DK + 1], BF16, tag=f"v{h}_{mt}", name=f"v{h}_{mt}")
                for mt in range(NLT)] for h in range(2)]

        # ---- q/k projections ----
        with tc.tile_pool(name="xin", bufs=1) as xin, \
             tc.tile_pool(name="prjp", bufs=2, space="PSUM") as prjp:
            qxs, kxs = [], []
            for cc in range(8):
                qx = xin.tile([128, L], BF16, tag=f"qx{cc}", name=f"qx{cc}")
                kx = xin.tile([128, L], BF16, tag=f"kx{cc}", name=f"kx{cc}")
                nc.sync.dma_start(qx[:], d_qt[cc * 128:(cc + 1) * 128, :])
                nc.scalar.dma_start(kx[:], d_kt[cc * 128:(cc + 1) * 128, :])
                qxs.append(qx)
                kxs.append(kx)
            for lc4 in range(4):
                lsl = bass.ts(lc4, 512)
                qp = prjp.tile([128, 512], F32, tag="qp")
                kp = prjp.tile([128, 512], F32, tag="kp")
                for cc in range(8):
                    wsl = bass.ts(cc, DH)
                    nc.tensor.matmul(qp[:], wqt_s[:, wsl], qxs[cc][:, lsl],
                                     start=(cc == 0), stop=(cc == 7))
                    nc.tensor.matmul(kp[:], wkt_s[:, wsl], kxs[cc][:, lsl],
                                     start=(cc == 0), stop=(cc == 7))
                for h in range(2):
                    hsl = slice(h * DK, (h + 1) * DK)
                    nc.scalar.activation(quv[h][0:DK, lsl], qp[hsl, :], AF.Identity,
                                         bias=ub_s[hsl, :], scale=0.125)
                    nc.scalar.activation(quv[h][DK:128, lsl], qp[hsl, :], AF.Identity,
                                         bias=vb_s[hsl, :], scale=0.125)
                    nc.scalar.activation(kth[h][:, lsl], kp[hsl, :], AF.Identity,
                                         bias=kb_s[hsl, :])

        # ---- v projection (natural layout, m on partitions) ----
        with tc.tile_pool(name="vin", bufs=1) as vin, \
             tc.tile_pool(name="vp", bufs=2, space="PSUM") as vp:
            vchunks = []
            for cc in range(8):
                vx = vin.tile([128, L], BF16, tag=f"vx{cc}")
                nc.sync.dma_start(vx[:], d_vt[cc * 128:(cc + 1) * 128, :])
                vchunks.append(vx)
            for mt in range(NLT):
                pv = vp.tile([128, DH], F32, tag="pv")
                for cc in range(8):
                    nc.tensor.matmul(pv[:], vchunks[cc][:, bass.ts(mt, 128)],
                                     wvt_s[:, bass.ts(cc, DH)],
                                     start=(cc == 0), stop=(cc == 7))
                for h in range(2):
                    nc.scalar.activation(vmt[h][mt][:, 0:DK],
                                         pv[:, h * DK:(h + 1) * DK], AF.Copy)
                    nc.gpsimd.memset(vmt[h][mt][:, DK:DK + 1], 1.0)

        # ---- attention + output ----
        ones2 = const.tile([2, DK], F32)
        nc.sync.dma_start(ones2[:], d_ones2[:])
        with tc.tile_pool(name="sc", bufs=3) as sc, \
             tc.tile_pool(name="pt", bufs=2) as ptp, \
             tc.tile_pool(name="at", bufs=2) as atp, \
             tc.tile_pool(name="acp", bufs=2, space="PSUM") as acp, \
             tc.tile_pool(name="bdp", bufs=2, space="PSUM") as bdp, \
             tc.tile_pool(name="pvp", bufs=1, space="PSUM") as pvp, \
             tc.tile_pool(name="oev", bufs=2) as oev:
            for lc in range(NLC):
                at_s = atp.tile([DH, LC], BF16, tag="at")
                for h in range(2):
                    pts = [ptp.tile([128, LC], BF16, tag=f"pt{mt}", name=f"pt{mt}")
                           for mt in range(NLT)]
                    for lt4 in range(NLC):
                        ltg = lc * 4 + lt4
                        l0 = ltg * LT
                        b0 = 1920 - l0
                        lhq = quv[h][0:DK, l0:l0 + LT]
                        lhv = quv[h][DK:128, l0:l0 + LT]
                        s_s = sc.tile([LT, L], BF16, tag="s")
                        band = sc.tile([LT, BW], BF16, tag="band")
                        # AC: 4 chunk matmuls, 1-bank psums
                        for q4 in range(4):
                            ap = acp.tile([LT, MC], F32, tag="ac")
                            nc.tensor.matmul(ap[:], lhq,
                                             kth[h][:, bass.ts(q4, MC)],
                                             start=True, stop=True)
                            dst = s_s[:, bass.ts(q4, MC)]
                            if q4 % 2 == 0:
                                nc.vector.tensor_copy(dst, ap[:])
                            else:
                                nc.scalar.activation(dst, ap[:], AF.Copy)
                        # band: 2x 1024-chunks + 128 tail
                        for half in range(2):
                            bp = bdp.tile([LT, 1024], F32, tag="bd")
                            for q2 in range(2):
                                fsl = slice(b0 + half * 1024 + q2 * 512,
                                            b0 + half * 1024 + (q2 + 1) * 512)
                                nc.tensor.matmul(bp[:, bass.ts(q2, MC)],
                                                 lhv, ft_s[DK:128, fsl],
                                                 start=True, stop=True)
                            dst = band[:, bass.ts(half, 1024)]
                            if half == 0:
                                nc.scalar.activation(dst, bp[:], AF.Copy)
                            else:
                                nc.vector.tensor_copy(dst, bp[:])
                        bt = bdp.tile([LT, 128], F32, tag="bd", name="bt")
                        nc.tensor.matmul(bt[:], lhv,
                                         ft_s[DK:128, b0 + 2048:b0 + BW],
                                         start=True, stop=True)
                        nc.scalar.activation(band[:, 2048:BW], bt[:], AF.Copy)
                        # diagonal shift-add: S[p, j] += band[p, 127-p+j]
                        diag = bass.AP(band[:].tensor, 127,
                                       [[BW - 1, LT], [1, L]])
                        nc.gpsimd.dma_start(s_s[:], diag,
                                            accum_op=mybir.AluOpType.add)
                        # exp
                        p_s = sc.tile([LT, L], BF16, tag="p")
                        nc.scalar.activation(p_s[:], s_s[:], AF.Exp)
                        # transpose blocks into per-m-tile P^T tiles
                        for mt in range(NLT):
                            nc.sync.dma_start_transpose(
                                pts[mt][:, bass.ts(lt4, LT)],
                                p_s[:, bass.ts(mt, LT)])
                    # PV: accumulate over m-tiles; row 64 = Z
                    po = pvp.tile([DK + 1, LC], F32, tag="po")
                    for mt in range(NLT):
                        nc.tensor.matmul(po[:], vmt[h][mt][:], pts[mt][:],
                                         start=(mt == 0), stop=(mt == NLT - 1))
                    # normalize: recipZ broadcast via K=2 ones-matmul
                    rz = oev.tile([2, LC], F32, tag="rz")
                    nc.gpsimd.memset(rz[:], 0.0)
                    nc.vector.reciprocal(rz[0:1, :], po[DK:DK + 1, :])
                    bc = acp.tile([DK, LC], F32, tag="ac")
                    nc.tensor.matmul(bc[:], ones2[:], rz[:],
                                     start=True, stop=True)
                    bcs = oev.tile([DK, LC], F32, tag="bcs")
                    nc.scalar.activation(bcs[:], bc[:], AF.Copy)
                    nc.vector.tensor_tensor(at_s[h * DK:(h + 1) * DK, :],
                                            po[0:DK, :], bcs[:],
                                            mybir.AluOpType.mult)
                # Wo: 8 e-tiles
                for et in range(8):
                    wp = acp.tile([128, LC], F32, tag="ac")
                    nc.tensor.matmul(wp[:], wot_s[:, bass.ts(et, 128)],
                                     at_s[:], start=True, stop=True)
                    osb = oev.tile([128, LC], F32, tag="osb")
                    if et % 2 == 0:
                        nc.scalar.activation(osb[:], wp[:], AF.Copy)
                    else:
                        nc.vector.tensor_copy(osb[:], wp[:])
                    nc.sync.dma_start(
                        d_out[et * 128:(et + 1) * 128, bass.ts(lc, LC)],
                        osb[:])
    nc.compile()
    return nc


_MODULE_CACHE = {}


def _get_module():
    if "nc" not in _MODULE_CACHE:
        _MODULE_CACHE["nc"] = _build_module()
    return _MODULE_CACHE["nc"]


def kernel(**inputs) -> np.ndarray:
    Q = np.asarray(inputs["Q"], np.float32)[0]      # (L, D)
    K = np.asarray(inputs["K"], np.float32)[0]
    V = np.asarray(inputs["V"], np.float32)[0]
    Wq = np.asarray(inputs["Wq"], np.float32)
    Wk = np.asarray(inputs["Wk"], np.float32)
    Wv = np.asarray(inputs["Wv"], np.float32)
    Wo = np.asarray(inputs["Wo"], np.float32)
    bq = np.asarray(inputs["bq"], np.float32)
    bk = np.asarray(inputs["bk"], np.float32)
    bv = np.asarray(inputs["bv"], np.float32)
    bo = np.asarray(inputs["bo"], np.float32)
    E = np.asarray(inputs["rel_emb"], np.float32)   # (4096, 64)
    u_b = np.asarray(inputs["u_bias"], np.float32)  # (16, 64)
    v_b = np.asarray(inputs["v_bias"], np.float32)

    bf = ml_dtypes.bfloat16
    QT = np.ascontiguousarray(Q.T).astype(bf)
    KT = np.ascontiguousarray(K.T).astype(bf)
    VT = np.ascontiguousarray(V.T).astype(bf)
    FT = np.ascontiguousarray(E[::-1].T).astype(bf)  # (64, 4096)

    in_maps = []
    for c in range(NCORES):
        ds = slice(DH * c, DH * c + DH)
        urep = np.concatenate([u_b[2 * c], u_b[2 * c + 1]])[:, None]
        vrep = np.concatenate([v_b[2 * c], v_b[2 * c + 1]])[:, None]
        in_maps.append({
            "qt": QT, "kt": KT, "vt": VT, "ft": FT,
            "wqt": np.ascontiguousarray(Wq[ds].T).astype(bf),
            "wkt": np.ascontiguousarray(Wk[ds].T).astype(bf),
            "wvt": np.ascontiguousarray(Wv[ds].T).astype(bf),
            "wot": np.ascontiguousarray(Wo[:, ds].T).astype(bf),
            "ubias": ((bq[ds, None] + urep) / 8.0).astype(np.float32),
            "vbias": ((bq[ds, None] + vrep) / 8.0).astype(np.float32),
            "kbias": bk[ds, None].astype(np.float32),
            "ones2": np.stack([np.ones(DK, np.float32),
                               np.zeros(DK, np.float32)]),
        })

    global _LAST_IN_MAPS
    _LAST_IN_MAPS = in_maps
    nc = _get_module()
    res = run_bass_kernel_spmd(nc, in_maps, core_ids=list(range(NCORES)))
    acc = np.zeros((D, L), np.float64)
    for r in res.results:
        acc += r["opart"].astype(np.float64)
    out = acc.T.astype(np.float32) + bo[None, :] + (bv @ Wo.T)[None, :]
    return out[None, :, :]

